# revision 15
# baseline (speedup 1.0000x reference)
"""Binarized LeNet5+BN forward on 8 Trainium2 NeuronCores.

Strategy (data-parallel over batch, 1024 images/core):
  * Everything on-chip is feature-major: [features on partitions, batch on free dim].
  * sign(x) / sign(hardtanh(bn(z))) == sign(scale*z + bias): each layer is
    matmul-accumulate into PSUM followed by ONE ScalarE activation
    (Sign with per-partition scale/bias = fused conv-bias+BN+hardtanh+sign).
  * The input is signed + laid out feature-major on the HOST (pointwise prep,
    like the host-built Toeplitz weight matrices): 4 phase-shifted fp8 copies
    [4,128,7,B] so every conv1 operand sits at partition base 0.
  * Convs are row-Toeplitz matmuls: weights are +-1/0 matrices built host-side;
    activations are y-major with image rows padded to 32 partitions. Conv output
    row y contracts rows [32y, 32y+160) of the previous layer.
  * All conv/fc1 matmul operands are fp8e4 (+-1/0 exact; PSUM accumulates fp32)
    with perf_mode=DoubleRow: each matmul contracts TWO K<=128 tiles (slices of
    the activation tensor paired along the free dim), halving PE passes.
    Measured DR cadence equals a plain N=512 matmul => 2x throughput.
  * conv1's 16-row remainder outputs land in one PSUM bank at 16-feature
    column offsets (odd offsets via a zero-padded 32-wide weight + accumulate),
    so ONE activation per 8 rows writes the 16-stride remainder bundle
    directly. Bundles live in the same tensor as the main activations (slots
    24+3p+s, 8 phase-shifted copies via SBUF DMA), so conv2 needs only THREE
    DoubleRow matmuls per (y2, Mt): (ky0,ky1), (ky2,ky3), (ky4, remainder).
    Padding lanes produce sign(garbage)=+-1 nulled by zero weight rows.
  * All weights/scales are packed into 3 DMA loads issued AFTER the first
    input tile loads, so the PE starts ~8us into the kernel.
  * Double-buffered pools let chunk c+1's input DMAs run under chunk c's
    matmuls, keeping the PE dense and the HAM clock un-throttled.
  * fc1 weights are column-permuted host-side to match the on-chip act2 layout;
    the final output stays feature-major [10, B] and is transposed on host.
"""

from contextlib import ExitStack

import ml_dtypes
import numpy as np

import concourse.bacc as bacc
import concourse.tile as tile
from concourse import mybir
from concourse.bass_utils import run_bass_kernel_spmd

F32 = mybir.dt.float32
BF16 = mybir.dt.bfloat16
FP8 = mybir.dt.float8e4
DR = mybir.MatmulPerfMode.DoubleRow
AF = mybir.ActivationFunctionType
EPS = np.float32(1e-5)
N_CORES = 8
B_TOTAL = 8192
B_CORE = B_TOTAL // N_CORES
CHUNK = 512
N_CHUNKS = B_CORE // CHUNK

_f8 = lambda a: np.ascontiguousarray(a.astype(ml_dtypes.float8_e4m3fn))
_bf = lambda a: np.ascontiguousarray(a.astype(ml_dtypes.bfloat16))
_f32c = lambda a: np.ascontiguousarray(a.astype(np.float32))

# wpack column offsets (fp8 [128, 2304])
_WOFF = {"w1p0": 0, "w1p1": 256, "w1ra": 288, "w1rb": 304, "w1roa": 320,
         "w1rob": 352, "w201": 384, "w223": 1024, "w24r": 1664}
_MTOFF = [0, 256, 512]  # per-Mt offsets inside a 640-col pair-set (2*Mp cols)


def _sign(a):
    return np.sign(a).astype(np.float32)


def _toeplitz1(w1s):  # [6,1,5,5] -> [160,144] rows (ky, xi<32), cols (c1,xo)
    W = np.zeros((160, 144), np.float32)
    xo = np.arange(24)
    for ky in range(5):
        for kx in range(5):
            for c1 in range(6):
                W[ky * 32 + xo + kx, c1 * 24 + xo] = w1s[c1, 0, ky, kx]
    return W


def _toeplitz2(w2s):
    """[16,6,5,5] -> main [128,5,320] rows (c1,xi24 mod 128), cols (c2,xo);
    remainder (last 16 rows of each 144-block) at 16-stride: [128,320]."""
    W = np.zeros((720, 320), np.float32)
    xo = np.arange(20)
    for ky in range(5):
        for c1 in range(6):
            for kx in range(5):
                for c2 in range(16):
                    W[ky * 144 + c1 * 24 + xo + kx, c2 * 20 + xo] = w2s[c2, c1, ky, kx]
    main = np.stack([W[144 * k : 144 * k + 128] for k in range(5)], 1)  # [128,5,320]
    rem16 = np.zeros((128, 320), np.float32)  # rows 16k+r (k<5)
    for k in range(5):
        rem16[16 * k : 16 * k + 16] = W[144 * k + 128 : 144 * k + 144]
    return main, rem16


def _affine(g, b, m, v, extra_bias):
    inv = (g.astype(np.float32) / np.sqrt(v.astype(np.float32) + EPS)).astype(np.float32)
    return inv, (inv * (extra_bias.astype(np.float32) - m.astype(np.float32)) + b.astype(np.float32)).astype(np.float32)


def build_consts(inp):
    """Host-side preprocessing of all weights/BN params into device constants."""
    C = {}
    W1 = _toeplitz1(_sign(inp["conv1_w"]))
    w2main, w2r16 = _toeplitz2(_sign(inp["conv2_w"]))
    wpack = np.zeros((128, 2304), np.float32)

    def put(name, arr):  # arr [p, cols]
        o = _WOFF[name]
        wpack[0 : arr.shape[0], o : o + arr.shape[1]] = arr

    def pair(a, b):  # [128, M] + [<=128, M] -> [128, 2M] interleaved pair-major
        out = np.zeros((128, 2, a.shape[1]), np.float32)
        out[:, 0, :] = a
        out[0 : b.shape[0], 1, :] = b
        return out.reshape(128, -1)

    put("w1p0", pair(W1[0:128, 0:128], W1[128:160, 0:128]))
    put("w1p1", pair(W1[0:128, 128:144], W1[128:160, 128:144]))
    put("w1ra", W1[0:128, 128:144])
    put("w1rb", W1[128:160, 128:144])
    w1ro = np.zeros((160, 32), np.float32)
    w1ro[:, 16:32] = W1[:, 128:144]  # odd 16-offset: left half zero
    put("w1roa", w1ro[0:128])
    put("w1rob", w1ro[128:160])
    for Mt in range(3):
        Mp = 128 if Mt < 2 else 64
        ms = slice(128 * Mt, 128 * Mt + Mp)
        wpack[:, _WOFF["w201"] + _MTOFF[Mt] : _WOFF["w201"] + _MTOFF[Mt] + 2 * Mp] = \
            pair(w2main[:, 0, ms], w2main[:, 1, ms])
        wpack[:, _WOFF["w223"] + _MTOFF[Mt] : _WOFF["w223"] + _MTOFF[Mt] + 2 * Mp] = \
            pair(w2main[:, 2, ms], w2main[:, 3, ms])
        wpack[:, _WOFF["w24r"] + _MTOFF[Mt] : _WOFF["w24r"] + _MTOFF[Mt] + 2 * Mp] = \
            pair(w2main[:, 4, ms], w2r16[:, ms])
    C["wpack"] = _f8(wpack)

    # fc1, permuted to the on-chip act2 layout: block b=(Mt*20+y2), row p -> feature
    # f2 = 320*y2 + 128*Mt + p  (= c2*20+xo within the y-block), orig col c2*400+y2*20+xo
    w3s = _sign(inp["fc1_w"])  # [120, 6400]
    W3Tp = np.zeros((128, 60, 128), np.float32)  # M padded 120->128 (DR stride % 16)
    for Mt in range(3):
        Kj = 128 if Mt < 2 else 64
        m = np.arange(Kj) + 128 * Mt
        c2, xo = m // 20, m % 20
        for y2 in range(20):
            cols = c2 * 400 + y2 * 20 + xo
            W3Tp[:Kj, Mt * 20 + y2, 0:120] = w3s[:, cols].T
    C["w3t"] = _f8(W3Tp)
    w45 = np.zeros((120, 94), np.float32)
    w45[0:120, 0:84] = _sign(inp["fc2_w"]).T
    w45[0:84, 84:94] = _sign(inp["fc3_w"]).T
    C["w45"] = _bf(w45)

    s1, b1 = _affine(inp["bn1_g"], inp["bn1_b"], inp["bn1_m"], inp["bn1_v"], inp["conv1_b"])
    s2, b2 = _affine(inp["bn2_g"], inp["bn2_b"], inp["bn2_m"], inp["bn2_v"], inp["conv2_b"])
    s3, b3 = _affine(inp["bnf1_g"], inp["bnf1_b"], inp["bnf1_m"], inp["bnf1_v"], inp["fc1_b"])
    s4, b4 = _affine(inp["bnf2_g"], inp["bnf2_b"], inp["bnf2_m"], inp["bnf2_v"], inp["fc2_b"])
    s5, b5 = _affine(inp["bnf3_g"], inp["bnf3_b"], inp["bnf3_m"], inp["bnf3_v"], inp["fc3_b"])
    c1v = np.arange(144) // 24
    sc1f, bi1f = s1[c1v], b1[c1v]
    c2v = np.arange(320) // 20
    sc2f, bi2f = s2[c2v], b2[c2v]
    scp = np.zeros((128, 16), np.float32)
    scp[:, 0], scp[:, 1] = sc1f[:128], bi1f[:128]
    for k in range(8):  # remainder scale/bias at 16-stride
        scp[16 * k : 16 * k + 16, 2] = sc1f[128:]
        scp[16 * k : 16 * k + 16, 3] = bi1f[128:]
    scp[:, 4], scp[:, 5] = sc2f[0:128], bi2f[0:128]
    scp[:, 6], scp[:, 7] = sc2f[128:256], bi2f[128:256]
    scp[0:64, 8], scp[0:64, 9] = sc2f[256:320], bi2f[256:320]
    scp[0:120, 10], scp[0:120, 11] = s3, b3
    scp[0:84, 12], scp[0:84, 13] = s4, b4
    scp[0:10, 14], scp[0:10, 15] = s5, b5
    C["scp"] = _f32c(scp)
    return C


def prep_x(x):
    """sign + feature-major layout + 4 phase shifts: [B,1,28,28] ->
    per-core [4, 128, 7, B_CORE] fp8 (xT row 32y+x = sign(img[y,x]), x<28)."""
    xs = np.sign(x.reshape(B_TOTAL, 28, 28)).astype(np.float32)
    res = []
    for i in range(N_CORES):
        xc = xs[i * B_CORE : (i + 1) * B_CORE]  # [b, 28, 28]
        tmp = np.zeros((B_CORE, 28, 32), np.float32)
        tmp[:, :, 0:28] = xc
        xT = np.zeros((1024, B_CORE), np.float32)
        xT[0:896] = tmp.reshape(B_CORE, 896).T
        xq = np.stack([xT[32 * q : 32 * q + 896].reshape(7, 128, B_CORE).transpose(1, 0, 2)
                       for q in range(4)])  # [4,128,7,b]
        res.append(_f8(xq))
    return res


def build_nc(consts, b_core=B_CORE, chunk=CHUNK, stage=99):
    n_chunks = b_core // chunk
    assert chunk % 128 == 0
    nc = bacc.Bacc(None, target_bir_lowering=False, debug=False)
    xt_in = nc.declare_dram_parameter("xt", [4, 128, 7, b_core], FP8, isOutput=False)
    if stage >= 37:
        out = nc.declare_dram_parameter("out", [10, b_core], F32, isOutput=True)
    else:
        dbg = nc.declare_dram_parameter("dbg", [128, 512], F32, isOutput=True)
    dr = {k: nc.inline_tensor(v, name=f"c_{k}") for k, v in consts.items()}

    with tile.TileContext(nc) as tc, ExitStack() as ctx:
        cp = ctx.enter_context(tc.tile_pool(name="consts", bufs=1))
        xtpool = ctx.enter_context(tc.tile_pool(name="xtpool", bufs=2))
        tpp = ctx.enter_context(tc.tile_pool(name="tp", bufs=1, space="PSUM"))
        cps = ctx.enter_context(tc.tile_pool(name="cps", bufs=2, space="PSUM"))
        fcp = ctx.enter_context(tc.tile_pool(name="fcp", bufs=1, space="PSUM"))
        apool = ctx.enter_context(tc.tile_pool(name="apool", bufs=2))
        a2pool = ctx.enter_context(tc.tile_pool(name="a2pool", bufs=2))
        fpool = ctx.enter_context(tc.tile_pool(name="fpool", bufs=2))
        dpool = ctx.enter_context(tc.tile_pool(name="dpool", bufs=2))

        def load_x(c, split=False):
            xtq = [xtpool.tile([128, 7, chunk], FP8, tag=f"xt{q}", name=f"xt{q}")
                   for q in range(4)]
            halves = [(0, 5), (5, 7)] if split else [(0, 7)]
            for lo, hi in halves:
                for q in range(4):
                    nc.sync.dma_start(out=xtq[q][:, lo:hi, :],
                                      in_=xt_in[q, :, lo:hi, c * chunk : (c + 1) * chunk])
            return xtq

        def cload(name, shape, dtype=FP8):
            t = cp.tile(shape, dtype, tag=f"c_{name}", name=f"c_{name}")
            nc.sync.dma_start(out=t[:], in_=dr[name][:])
            return t

        # issue order: first input slots -> conv1 weights/scales -> the rest
        xtq_next = [xtpool.tile([128, 7, chunk], FP8, tag=f"xt{q}", name=f"xt{q}")
                    for q in range(4)]
        for q in range(4):
            nc.sync.dma_start(out=xtq_next[q][:, 0:5, :], in_=xt_in[q, :, 0:5, 0:chunk])
        wp = cload("wpack", [128, 2304])
        scp = cload("scp", [128, 16], F32)
        for q in range(4):
            nc.sync.dma_start(out=xtq_next[q][:, 5:7, :], in_=xt_in[q, :, 5:7, 0:chunk])
        w3t = cload("w3t", [128, 60, 128])
        w45 = cload("w45", [120, 94], BF16)

        def wdr(name, Mt=None):  # DoubleRow pair view [128, 2, Mp]
            if Mt is None:
                o, Mp = _WOFF[name], {"w1p0": 128, "w1p1": 16}[name]
            else:
                o, Mp = _WOFF[name] + _MTOFF[Mt], 128 if Mt < 2 else 64
            return wp[:, o : o + 2 * Mp].rearrange("p (a m) -> p a m", a=2)

        def wsl(name, p, cols):
            o = _WOFF[name]
            return wp[0:p, o : o + cols]

        def scb(col, p):  # (scale, bias) column pair from scp
            return scp[0:p, col : col + 1], scp[0:p, col + 1 : col + 2]

        for c in range(n_chunks):
            xtq = xtq_next
            if stage <= 1:
                dt_ = dpool.tile([128, 512], F32, tag="dbg")
                nc.vector.tensor_copy(out=dt_[:], in_=xtq[1][:, 0, 0:512])
                nc.sync.dma_start(out=dbg[:], in_=dt_[:])
                continue

            # ---- conv1: 1 DoubleRow matmul per (y1, Mt-main); the 16-feature
            # remainder accumulates into c1r at column offset 16*(y1%8) (odd
            # offsets via the zero-left-half 32-wide weight), one act per 8 y1.
            # actc slots: 0..23 main y rows; 24+3p+s = remainder bundles.
            actc = apool.tile([128, 48, chunk], FP8, tag="actc")
            for yg in range(12):
                ps0 = cps.tile([128, 2, chunk], F32, tag="cps")
                if yg % 4 == 0:
                    c1r = fcp.tile([128, chunk], F32, tag="c1r", bufs=2)
                j = yg % 4
                for ty in range(2):
                    y1 = 2 * yg + ty
                    q, t = y1 % 4, y1 // 4
                    nc.tensor.matmul(ps0[:, ty, :], wdr("w1p0"), xtq[q][:, t : t + 2, :],
                                     start=True, stop=True, perf_mode=DR)
                # remainder: odd y1 (32-wide zero-padded weight) first, then even
                yo, ye = 2 * yg + 1, 2 * yg
                qo, to = yo % 4, yo // 4
                qe, te = ye % 4, ye // 4
                nc.tensor.matmul(c1r[32 * j : 32 * j + 32, :], wsl("w1roa", 128, 32),
                                 xtq[qo][:, to, :], start=True, stop=False,
                                 tile_position=(0, 32 * j))
                nc.tensor.matmul(c1r[32 * j : 32 * j + 32, :], wsl("w1rob", 32, 32),
                                 xtq[qo][0:32, to + 1, :], start=False, stop=False,
                                 tile_position=(0, 32 * j))
                nc.tensor.matmul(c1r[32 * j : 32 * j + 16, :], wsl("w1ra", 128, 16),
                                 xtq[qe][:, te, :], start=False, stop=False,
                                 tile_position=(0, 32 * j))
                nc.tensor.matmul(c1r[32 * j : 32 * j + 16, :], wsl("w1rb", 32, 16),
                                 xtq[qe][0:32, te + 1, :], start=False, stop=True,
                                 tile_position=(0, 32 * j))
                s0, b0 = scb(0, 128)
                nc.scalar.activation(actc[:, 2 * yg : 2 * yg + 2, :], ps0[:], AF.Sign,
                                     bias=b0, scale=s0)
                # Mt=1 main output = the same 16 remainder features; they are
                # produced by the c1r path, so no separate w1p1 matmul needed.
                if yg % 4 == 3:
                    s1_, b1_ = scb(2, 128)
                    nc.scalar.activation(actc[:, 24 + yg // 4, :], c1r[:], AF.Sign,
                                         bias=b1_, scale=s1_)
            # 7 phase-shifted copies of the remainder bundles (16-row shifts)
            for p in range(1, 8):
                ns = 3 if p <= 3 else 2
                if p <= 3:
                    nc.vector.memset(actc[:, 24 + 3 * p + 2, :], 0.0)
                nc.sync.dma_start(out=actc[0 : 128 - 16 * p, 24 + 3 * p : 24 + 3 * p + ns, :],
                                  in_=actc[16 * p : 128, 24 : 24 + ns, :])
                nc.sync.dma_start(out=actc[128 - 16 * p : 128, 24 + 3 * p : 24 + 3 * p + 2, :],
                                  in_=actc[0 : 16 * p, 25 : 27, :])
            if c + 1 < n_chunks:  # issue next chunk's input loads early
                xtq_next = load_x(c + 1)
            if stage <= 2:
                dt_ = dpool.tile([128, 512], F32, tag="dbg")
                nc.vector.tensor_copy(out=dt_[:], in_=actc[:, 0, 0:512])
                nc.sync.dma_start(out=dbg[:], in_=dt_[:])
                continue

            # ---- conv2 (Toeplitz over actc): THREE DR matmuls per (y2, Mt);
            # fc1's 30 accumulating DR matmuls interleave as act2 slots land ----
            act2 = a2pool.tile([128, 3, 20, chunk], FP8, tag="act2")
            f1ps = fcp.tile([128, chunk], F32, tag="f1ps")
            PAIRS = [(0, 8), (1, 9), (2, 10), (3, 11), (4, 12), (5, 13),
                     (6, 14), (7, 15), (16, 18), (17, 19)]
            fc1_first = True
            for pi, (ya, yb) in enumerate(PAIRS):
                pss = []
                for Mt in range(3):
                    Mp = 128 if Mt < 2 else 64
                    ps = cps.tile([Mp, 2, chunk], F32, tag="cps", name=f"c2ps{Mt}")
                    pss.append((ps, Mp))
                    for ty, y2 in ((0, ya), (1, yb)):
                        p8, s8 = y2 % 8, y2 // 8
                        d = (24 + 3 * p8 + s8) - (y2 + 4)
                        nc.tensor.matmul(ps[0:Mp, ty, :], wdr("w201", Mt),
                                         actc[:, y2 : y2 + 2, :],
                                         start=True, stop=False, perf_mode=DR)
                        nc.tensor.matmul(ps[0:Mp, ty, :], wdr("w223", Mt),
                                         actc[:, y2 + 2 : y2 + 4, :],
                                         start=False, stop=False, perf_mode=DR)
                        nc.tensor.matmul(ps[0:Mp, ty, :], wdr("w24r", Mt),
                                         actc[:, y2 + 4 : y2 + 5 + d : d, :],
                                         start=False, stop=True, perf_mode=DR)
                for Mt, (ps, Mp) in enumerate(pss):
                    s2_, b2_ = scb(4 + 2 * Mt, Mp)
                    nc.scalar.activation(act2[0:Mp, Mt, ya : yb + 1 : yb - ya, :], ps[:],
                                         AF.Sign, bias=b2_, scale=s2_)
                if pi % 2 == 1 and pi >= 3:  # completed 2 pairs ago (act slack)
                    ys = (pi - 3, pi + 5)
                    for Mt in range(3):
                        Kj = 128 if Mt < 2 else 64
                        for y2 in ys:
                            b = Mt * 20 + y2
                            nc.tensor.matmul(f1ps[:], w3t[0:Kj, b : b + 2, :],
                                             act2[0:Kj, Mt, y2 : y2 + 2, :],
                                             start=fc1_first, stop=(b == 58),
                                             perf_mode=DR)
                            fc1_first = False
            for y2 in (16, 18):  # final fc1 round (slots from pairs 8, 9)
                for Mt in range(3):
                    Kj = 128 if Mt < 2 else 64
                    b = Mt * 20 + y2
                    nc.tensor.matmul(f1ps[:], w3t[0:Kj, b : b + 2, :],
                                     act2[0:Kj, Mt, y2 : y2 + 2, :],
                                     start=False, stop=(b == 58), perf_mode=DR)
            if stage <= 3:
                dt_ = dpool.tile([128, 512], F32, tag="dbg")
                nc.vector.tensor_copy(out=dt_[:], in_=act2[:, 0, 0, 0:512])
                nc.sync.dma_start(out=dbg[:], in_=dt_[:])
                continue

            a3 = fpool.tile([120, chunk], BF16, tag="a3")
            s3_, b3_ = scb(10, 120)
            nc.scalar.activation(a3[:], f1ps[0:120, :], AF.Sign, bias=b3_, scale=s3_)
            if stage <= 35:
                dt_ = dpool.tile([128, 512], F32, tag="dbg")
                nc.any.memset(dt_[:], 0.0)
                nc.vector.tensor_copy(out=dt_[0:120, :], in_=a3[:, 0:512])
                nc.sync.dma_start(out=dbg[:], in_=dt_[:])
                continue
            f2ps = tpp.tile([84, chunk], F32, tag="tp")
            nc.tensor.matmul(f2ps[:], w45[0:120, 0:84], a3[:], start=True, stop=True)
            a4 = fpool.tile([84, chunk], BF16, tag="a4")
            s4_, b4_ = scb(12, 84)
            nc.scalar.activation(a4[:], f2ps[:], AF.Sign, bias=b4_, scale=s4_)
            f3ps = tpp.tile([10, chunk], F32, tag="tp")
            nc.tensor.matmul(f3ps[:], w45[0:84, 84:94], a4[:], start=True, stop=True)
            o5 = fpool.tile([10, chunk], F32, tag="o5")
            s5_, b5_ = scb(14, 10)
            nc.scalar.activation(o5[:], f3ps[:], AF.Identity, bias=b5_, scale=s5_)
            if stage <= 36:
                dt_ = dpool.tile([128, 512], F32, tag="dbg")
                nc.any.memset(dt_[:], 0.0)
                nc.vector.tensor_copy(out=dt_[0:10, :], in_=o5[:, 0:512])
                nc.sync.dma_start(out=dbg[:], in_=dt_[:])
                continue
            nc.sync.dma_start(out=out[:, c * chunk : (c + 1) * chunk], in_=o5[:])

    nc.compile()
    return nc


def kernel(**inputs):
    inputs = {k: np.asarray(v) for k, v in inputs.items()}
    consts = build_consts(inputs)
    nc = build_nc(consts)
    xs = prep_x(inputs["x"].astype(np.float32))
    in_maps = [{"xt": xs[i]} for i in range(N_CORES)]
    res = run_bass_kernel_spmd(nc, in_maps, core_ids=list(range(N_CORES)))
    out = np.concatenate([np.asarray(r["out"]).astype(np.float32).T for r in res.results], axis=0)
    return out.astype(np.float32)


# revision 16
# speedup vs baseline: 1.1239x; 1.1239x over previous
"""Binarized LeNet5+BN forward on 8 Trainium2 NeuronCores.

Strategy (data-parallel over batch, 1024 images/core):
  * Everything on-chip is feature-major: [features on partitions, batch on free dim].
  * sign(x) / sign(hardtanh(bn(z))) == sign(scale*z + bias): each layer is
    matmul-accumulate into PSUM followed by ONE ScalarE activation
    (Sign with per-partition scale/bias = fused conv-bias+BN+hardtanh+sign).
  * The input is signed + laid out feature-major on the HOST (pointwise prep,
    like the host-built Toeplitz weight matrices): 4 phase-shifted fp8 copies
    [4,128,7,B] so every conv1 operand sits at partition base 0.
  * Convs are row-Toeplitz matmuls: weights are +-1/0 matrices built host-side;
    activations are y-major with image rows padded to 32 partitions. Conv output
    row y contracts rows [32y, 32y+160) of the previous layer.
  * All conv/fc1 matmul operands are fp8e4 (+-1/0 exact; PSUM accumulates fp32)
    with perf_mode=DoubleRow: each matmul contracts TWO K<=128 tiles (slices of
    the activation tensor paired along the free dim), halving PE passes.
    Measured DR cadence equals a plain N=512 matmul => 2x throughput.
  * conv1's 16-row remainder outputs land in one PSUM bank at 16-feature
    column offsets (odd offsets via a zero-padded 32-wide weight + accumulate),
    so ONE activation per 8 rows writes the 16-stride remainder bundle
    directly. Bundles live in the same tensor as the main activations (slots
    24+3p+s, 8 phase-shifted copies via SBUF DMA), so conv2 needs only THREE
    DoubleRow matmuls per (y2, Mt): (ky0,ky1), (ky2,ky3), (ky4, remainder).
    Padding lanes produce sign(garbage)=+-1 nulled by zero weight rows.
  * All weights/scales are packed into 3 DMA loads issued AFTER the first
    input tile loads, so the PE starts ~8us into the kernel.
  * Double-buffered pools let chunk c+1's input DMAs run under chunk c's
    matmuls, keeping the PE dense and the HAM clock un-throttled.
  * fc1 weights are column-permuted host-side to match the on-chip act2 layout;
    the final output stays feature-major [10, B] and is transposed on host.
"""

from contextlib import ExitStack

import ml_dtypes
import numpy as np

import concourse.bacc as bacc
import concourse.tile as tile
from concourse import mybir
from concourse.bass_utils import run_bass_kernel_spmd

F32 = mybir.dt.float32
BF16 = mybir.dt.bfloat16
FP8 = mybir.dt.float8e4
DR = mybir.MatmulPerfMode.DoubleRow
AF = mybir.ActivationFunctionType
EPS = np.float32(1e-5)
N_CORES = 8
B_TOTAL = 8192
B_CORE = B_TOTAL // N_CORES
CHUNK = 512
N_CHUNKS = B_CORE // CHUNK

_f8 = lambda a: np.ascontiguousarray(a.astype(ml_dtypes.float8_e4m3fn))
_bf = lambda a: np.ascontiguousarray(a.astype(ml_dtypes.bfloat16))
_f32c = lambda a: np.ascontiguousarray(a.astype(np.float32))

# wpack column offsets (fp8 [128, 2304])
_WOFF = {"w1p0": 0, "w1p1": 256, "w1ra": 288, "w1rb": 304, "w1roa": 320,
         "w1rob": 352, "w201": 384, "w223": 1024, "w24r": 1664}
_MTOFF = [0, 256, 512]  # per-Mt offsets inside a 640-col pair-set (2*Mp cols)


def _sign(a):
    return np.sign(a).astype(np.float32)


def _toeplitz1(w1s):  # [6,1,5,5] -> [160,144] rows (ky, xi<32), cols (c1,xo)
    W = np.zeros((160, 144), np.float32)
    xo = np.arange(24)
    for ky in range(5):
        for kx in range(5):
            for c1 in range(6):
                W[ky * 32 + xo + kx, c1 * 24 + xo] = w1s[c1, 0, ky, kx]
    return W


def _toeplitz2(w2s):
    """[16,6,5,5] -> main [128,5,320] rows (c1,xi24 mod 128), cols (c2,xo);
    remainder (last 16 rows of each 144-block) at 16-stride: [128,320]."""
    W = np.zeros((720, 320), np.float32)
    xo = np.arange(20)
    for ky in range(5):
        for c1 in range(6):
            for kx in range(5):
                for c2 in range(16):
                    W[ky * 144 + c1 * 24 + xo + kx, c2 * 20 + xo] = w2s[c2, c1, ky, kx]
    main = np.stack([W[144 * k : 144 * k + 128] for k in range(5)], 1)  # [128,5,320]
    rem16 = np.zeros((128, 320), np.float32)  # rows 16k+r (k<5)
    for k in range(5):
        rem16[16 * k : 16 * k + 16] = W[144 * k + 128 : 144 * k + 144]
    return main, rem16


def _affine(g, b, m, v, extra_bias):
    inv = (g.astype(np.float32) / np.sqrt(v.astype(np.float32) + EPS)).astype(np.float32)
    return inv, (inv * (extra_bias.astype(np.float32) - m.astype(np.float32)) + b.astype(np.float32)).astype(np.float32)


def build_consts(inp):
    """Host-side preprocessing of all weights/BN params into device constants."""
    C = {}
    W1 = _toeplitz1(_sign(inp["conv1_w"]))
    w2main, w2r16 = _toeplitz2(_sign(inp["conv2_w"]))
    wpack = np.zeros((128, 2304), np.float32)

    def put(name, arr):  # arr [p, cols]
        o = _WOFF[name]
        wpack[0 : arr.shape[0], o : o + arr.shape[1]] = arr

    def pair(a, b):  # [128, M] + [<=128, M] -> [128, 2M] interleaved pair-major
        out = np.zeros((128, 2, a.shape[1]), np.float32)
        out[:, 0, :] = a
        out[0 : b.shape[0], 1, :] = b
        return out.reshape(128, -1)

    put("w1p0", pair(W1[0:128, 0:128], W1[128:160, 0:128]))
    put("w1p1", pair(W1[0:128, 128:144], W1[128:160, 128:144]))
    put("w1ra", W1[0:128, 128:144])
    put("w1rb", W1[128:160, 128:144])
    w1ro = np.zeros((160, 32), np.float32)
    w1ro[:, 16:32] = W1[:, 128:144]  # odd 16-offset: left half zero
    put("w1roa", w1ro[0:128])
    put("w1rob", w1ro[128:160])
    for Mt in range(3):
        Mp = 128 if Mt < 2 else 64
        ms = slice(128 * Mt, 128 * Mt + Mp)
        wpack[:, _WOFF["w201"] + _MTOFF[Mt] : _WOFF["w201"] + _MTOFF[Mt] + 2 * Mp] = \
            pair(w2main[:, 0, ms], w2main[:, 1, ms])
        wpack[:, _WOFF["w223"] + _MTOFF[Mt] : _WOFF["w223"] + _MTOFF[Mt] + 2 * Mp] = \
            pair(w2main[:, 2, ms], w2main[:, 3, ms])
        wpack[:, _WOFF["w24r"] + _MTOFF[Mt] : _WOFF["w24r"] + _MTOFF[Mt] + 2 * Mp] = \
            pair(w2main[:, 4, ms], w2r16[:, ms])
    C["wpack"] = _f8(wpack)

    # fc1, permuted to the on-chip act2 layout: block b=(Mt*20+y2), row p -> feature
    # f2 = 320*y2 + 128*Mt + p  (= c2*20+xo within the y-block), orig col c2*400+y2*20+xo
    w3s = _sign(inp["fc1_w"])  # [120, 6400]
    W3Tp = np.zeros((128, 60, 128), np.float32)  # M padded 120->128 (DR stride % 16)
    for Mt in range(3):
        Kj = 128 if Mt < 2 else 64
        m = np.arange(Kj) + 128 * Mt
        c2, xo = m // 20, m % 20
        for y2 in range(20):
            cols = c2 * 400 + y2 * 20 + xo
            W3Tp[:Kj, Mt * 20 + y2, 0:120] = w3s[:, cols].T
    C["w3t"] = _f8(W3Tp)
    w45 = np.zeros((120, 94), np.float32)
    w45[0:120, 0:84] = _sign(inp["fc2_w"]).T
    w45[0:84, 84:94] = _sign(inp["fc3_w"]).T
    C["w45"] = _bf(w45)

    s1, b1 = _affine(inp["bn1_g"], inp["bn1_b"], inp["bn1_m"], inp["bn1_v"], inp["conv1_b"])
    s2, b2 = _affine(inp["bn2_g"], inp["bn2_b"], inp["bn2_m"], inp["bn2_v"], inp["conv2_b"])
    s3, b3 = _affine(inp["bnf1_g"], inp["bnf1_b"], inp["bnf1_m"], inp["bnf1_v"], inp["fc1_b"])
    s4, b4 = _affine(inp["bnf2_g"], inp["bnf2_b"], inp["bnf2_m"], inp["bnf2_v"], inp["fc2_b"])
    s5, b5 = _affine(inp["bnf3_g"], inp["bnf3_b"], inp["bnf3_m"], inp["bnf3_v"], inp["fc3_b"])
    c1v = np.arange(144) // 24
    sc1f, bi1f = s1[c1v], b1[c1v]
    c2v = np.arange(320) // 20
    sc2f, bi2f = s2[c2v], b2[c2v]
    scp = np.zeros((128, 16), np.float32)
    scp[:, 0], scp[:, 1] = sc1f[:128], bi1f[:128]
    for k in range(8):  # remainder scale/bias at 16-stride
        scp[16 * k : 16 * k + 16, 2] = sc1f[128:]
        scp[16 * k : 16 * k + 16, 3] = bi1f[128:]
    scp[:, 4], scp[:, 5] = sc2f[0:128], bi2f[0:128]
    scp[:, 6], scp[:, 7] = sc2f[128:256], bi2f[128:256]
    scp[0:64, 8], scp[0:64, 9] = sc2f[256:320], bi2f[256:320]
    scp[0:120, 10], scp[0:120, 11] = s3, b3
    scp[0:84, 12], scp[0:84, 13] = s4, b4
    scp[0:10, 14], scp[0:10, 15] = s5, b5
    C["scp"] = _f32c(scp)
    return C


def prep_x(x):
    """sign + feature-major layout + 4 phase shifts: [B,1,28,28] ->
    per-core [4, 128, 7, B_CORE] fp8 (xT row 32y+x = sign(img[y,x]), x<28)."""
    xs = np.sign(x.reshape(B_TOTAL, 28, 28)).astype(np.float32)
    res = []
    for i in range(N_CORES):
        xc = xs[i * B_CORE : (i + 1) * B_CORE]  # [b, 28, 28]
        tmp = np.zeros((B_CORE, 28, 32), np.float32)
        tmp[:, :, 0:28] = xc
        xT = np.zeros((1024, B_CORE), np.float32)
        xT[0:896] = tmp.reshape(B_CORE, 896).T
        xq = np.stack([xT[32 * q : 32 * q + 896].reshape(7, 128, B_CORE).transpose(1, 0, 2)
                       for q in range(4)])  # [4,128,7,b]
        res.append(_f8(xq))
    return res


def build_nc(consts, b_core=B_CORE, chunk=CHUNK, stage=99):
    n_chunks = b_core // chunk
    assert chunk % 128 == 0
    nc = bacc.Bacc(None, target_bir_lowering=False, debug=False)
    xt_in = nc.declare_dram_parameter("xt", [4, 128, 7, b_core], FP8, isOutput=False)
    if stage >= 37:
        out = nc.declare_dram_parameter("out", [10, b_core], F32, isOutput=True)
    else:
        dbg = nc.declare_dram_parameter("dbg", [128, 512], F32, isOutput=True)
    dr = {k: nc.inline_tensor(v, name=f"c_{k}") for k, v in consts.items()}

    with tile.TileContext(nc) as tc, ExitStack() as ctx:
        cp = ctx.enter_context(tc.tile_pool(name="consts", bufs=1))
        xtpool = ctx.enter_context(tc.tile_pool(name="xtpool", bufs=2))
        tpp = ctx.enter_context(tc.tile_pool(name="tp", bufs=2, space="PSUM"))
        cps = ctx.enter_context(tc.tile_pool(name="cps", bufs=2, space="PSUM"))
        fcp = ctx.enter_context(tc.tile_pool(name="fcp", bufs=1, space="PSUM"))
        apool = ctx.enter_context(tc.tile_pool(name="apool", bufs=2))
        a2pool = ctx.enter_context(tc.tile_pool(name="a2pool", bufs=2))
        fpool = ctx.enter_context(tc.tile_pool(name="fpool", bufs=2))
        dpool = ctx.enter_context(tc.tile_pool(name="dpool", bufs=2))

        def load_x(c, split=False):
            xtq = [xtpool.tile([128, 7, chunk], FP8, tag=f"xt{q}", name=f"xt{q}")
                   for q in range(4)]
            halves = [(0, 5), (5, 7)] if split else [(0, 7)]
            for lo, hi in halves:
                for q in range(4):
                    nc.sync.dma_start(out=xtq[q][:, lo:hi, :],
                                      in_=xt_in[q, :, lo:hi, c * chunk : (c + 1) * chunk])
            return xtq

        def cload(name, shape, dtype=FP8):
            t = cp.tile(shape, dtype, tag=f"c_{name}", name=f"c_{name}")
            nc.sync.dma_start(out=t[:], in_=dr[name][:])
            return t

        xtq_next = load_x(0)  # input loads issue before const loads
        wp = cload("wpack", [128, 2304])
        w3t = cload("w3t", [128, 60, 128])
        w45 = cload("w45", [120, 94], BF16)
        scp = cload("scp", [128, 16], F32)

        def wdr(name, Mt=None):  # DoubleRow pair view [128, 2, Mp]
            if Mt is None:
                o, Mp = _WOFF[name], {"w1p0": 128, "w1p1": 16}[name]
            else:
                o, Mp = _WOFF[name] + _MTOFF[Mt], 128 if Mt < 2 else 64
            return wp[:, o : o + 2 * Mp].rearrange("p (a m) -> p a m", a=2)

        def wsl(name, p, cols):
            o = _WOFF[name]
            return wp[0:p, o : o + cols]

        def scb(col, p):  # (scale, bias) column pair from scp
            return scp[0:p, col : col + 1], scp[0:p, col + 1 : col + 2]

        for c in range(n_chunks):
            xtq = xtq_next
            if stage <= 1:
                dt_ = dpool.tile([128, 512], F32, tag="dbg")
                nc.vector.tensor_copy(out=dt_[:], in_=xtq[1][:, 0, 0:512])
                nc.sync.dma_start(out=dbg[:], in_=dt_[:])
                continue

            # ---- conv1: 1 DoubleRow matmul per (y1, Mt-main); the 16-feature
            # remainder accumulates into c1r at column offset 16*(y1%8) (odd
            # offsets via the zero-left-half 32-wide weight), one act per 8 y1.
            # actc slots: 0..23 main y rows; 24+3p+s = remainder bundles.
            actc = apool.tile([128, 48, chunk], FP8, tag="actc")
            for yg in range(12):
                ps0 = cps.tile([128, 2, chunk], F32, tag="cps")
                if yg % 4 == 0:
                    c1r = fcp.tile([128, chunk], F32, tag="c1r")
                j = yg % 4
                for ty in range(2):
                    y1 = 2 * yg + ty
                    q, t = y1 % 4, y1 // 4
                    nc.tensor.matmul(ps0[:, ty, :], wdr("w1p0"), xtq[q][:, t : t + 2, :],
                                     start=True, stop=True, perf_mode=DR)
                # remainder: odd y1 (32-wide zero-padded weight) first, then even
                yo, ye = 2 * yg + 1, 2 * yg
                qo, to = yo % 4, yo // 4
                qe, te = ye % 4, ye // 4
                nc.tensor.matmul(c1r[32 * j : 32 * j + 32, :], wsl("w1roa", 128, 32),
                                 xtq[qo][:, to, :], start=True, stop=False,
                                 tile_position=(0, 32 * j))
                nc.tensor.matmul(c1r[32 * j : 32 * j + 32, :], wsl("w1rob", 32, 32),
                                 xtq[qo][0:32, to + 1, :], start=False, stop=False,
                                 tile_position=(0, 32 * j))
                nc.tensor.matmul(c1r[32 * j : 32 * j + 16, :], wsl("w1ra", 128, 16),
                                 xtq[qe][:, te, :], start=False, stop=False,
                                 tile_position=(0, 32 * j))
                nc.tensor.matmul(c1r[32 * j : 32 * j + 16, :], wsl("w1rb", 32, 16),
                                 xtq[qe][0:32, te + 1, :], start=False, stop=True,
                                 tile_position=(0, 32 * j))
                s0, b0 = scb(0, 128)
                nc.scalar.activation(actc[:, 2 * yg : 2 * yg + 2, :], ps0[:], AF.Sign,
                                     bias=b0, scale=s0)
                # Mt=1 main output = the same 16 remainder features; they are
                # produced by the c1r path, so no separate w1p1 matmul needed.
                if yg % 4 == 3:
                    s1_, b1_ = scb(2, 128)
                    nc.scalar.activation(actc[:, 24 + yg // 4, :], c1r[:], AF.Sign,
                                         bias=b1_, scale=s1_)
            # 7 phase-shifted copies of the remainder bundles (16-row shifts)
            for p in range(1, 8):
                ns = 3 if p <= 3 else 2
                if p <= 3:
                    nc.vector.memset(actc[:, 24 + 3 * p + 2, :], 0.0)
                nc.sync.dma_start(out=actc[0 : 128 - 16 * p, 24 + 3 * p : 24 + 3 * p + ns, :],
                                  in_=actc[16 * p : 128, 24 : 24 + ns, :])
                nc.sync.dma_start(out=actc[128 - 16 * p : 128, 24 + 3 * p : 24 + 3 * p + 2, :],
                                  in_=actc[0 : 16 * p, 25 : 27, :])
            if c + 1 < n_chunks:  # issue next chunk's input loads early
                xtq_next = load_x(c + 1)
            if stage <= 2:
                dt_ = dpool.tile([128, 512], F32, tag="dbg")
                nc.vector.tensor_copy(out=dt_[:], in_=actc[:, 0, 0:512])
                nc.sync.dma_start(out=dbg[:], in_=dt_[:])
                continue

            # ---- conv2 (Toeplitz over actc): THREE DR matmuls per (y2, Mt);
            # fc1's 30 accumulating DR matmuls interleave as act2 slots land ----
            act2 = a2pool.tile([128, 3, 20, chunk], FP8, tag="act2")
            PAIRS = [(0, 8), (1, 9), (2, 10), (3, 11), (4, 12), (5, 13),
                     (6, 14), (7, 15), (16, 18), (17, 19)]
            for pi, (ya, yb) in enumerate(PAIRS):
                pss = []
                for Mt in range(3):
                    Mp = 128 if Mt < 2 else 64
                    ps = cps.tile([Mp, 2, chunk], F32, tag="cps", name=f"c2ps{Mt}")
                    pss.append((ps, Mp))
                    for ty, y2 in ((0, ya), (1, yb)):
                        p8, s8 = y2 % 8, y2 // 8
                        d = (24 + 3 * p8 + s8) - (y2 + 4)
                        nc.tensor.matmul(ps[0:Mp, ty, :], wdr("w201", Mt),
                                         actc[:, y2 : y2 + 2, :],
                                         start=True, stop=False, perf_mode=DR)
                        nc.tensor.matmul(ps[0:Mp, ty, :], wdr("w223", Mt),
                                         actc[:, y2 + 2 : y2 + 4, :],
                                         start=False, stop=False, perf_mode=DR)
                        nc.tensor.matmul(ps[0:Mp, ty, :], wdr("w24r", Mt),
                                         actc[:, y2 + 4 : y2 + 5 + d : d, :],
                                         start=False, stop=True, perf_mode=DR)
                for Mt, (ps, Mp) in enumerate(pss):
                    s2_, b2_ = scb(4 + 2 * Mt, Mp)
                    nc.scalar.activation(act2[0:Mp, Mt, ya : yb + 1 : yb - ya, :], ps[:],
                                         AF.Sign, bias=b2_, scale=s2_)
            if stage <= 3:
                dt_ = dpool.tile([128, 512], F32, tag="dbg")
                nc.vector.tensor_copy(out=dt_[:], in_=act2[:, 0, 0, 0:512])
                nc.sync.dma_start(out=dbg[:], in_=dt_[:])
                continue

            f1ps = fcp.tile([128, chunk], F32, tag="f1ps")
            for Mt in range(3):
                Kj = 128 if Mt < 2 else 64
                for y2 in range(0, 20, 2):
                    b = Mt * 20 + y2
                    nc.tensor.matmul(f1ps[:], w3t[0:Kj, b : b + 2, :],
                                     act2[0:Kj, Mt, y2 : y2 + 2, :],
                                     start=(b == 0), stop=(b == 58), perf_mode=DR)
            a3 = fpool.tile([120, chunk], BF16, tag="a3")
            s3_, b3_ = scb(10, 120)
            nc.scalar.activation(a3[:], f1ps[0:120, :], AF.Sign, bias=b3_, scale=s3_)
            if stage <= 35:
                dt_ = dpool.tile([128, 512], F32, tag="dbg")
                nc.any.memset(dt_[:], 0.0)
                nc.vector.tensor_copy(out=dt_[0:120, :], in_=a3[:, 0:512])
                nc.sync.dma_start(out=dbg[:], in_=dt_[:])
                continue
            f2ps = tpp.tile([84, chunk], F32, tag="tp")
            nc.tensor.matmul(f2ps[:], w45[0:120, 0:84], a3[:], start=True, stop=True)
            a4 = fpool.tile([84, chunk], BF16, tag="a4")
            s4_, b4_ = scb(12, 84)
            nc.scalar.activation(a4[:], f2ps[:], AF.Sign, bias=b4_, scale=s4_)
            f3ps = tpp.tile([10, chunk], F32, tag="tp")
            nc.tensor.matmul(f3ps[:], w45[0:84, 84:94], a4[:], start=True, stop=True)
            o5 = fpool.tile([10, chunk], F32, tag="o5")
            s5_, b5_ = scb(14, 10)
            nc.scalar.activation(o5[:], f3ps[:], AF.Identity, bias=b5_, scale=s5_)
            if stage <= 36:
                dt_ = dpool.tile([128, 512], F32, tag="dbg")
                nc.any.memset(dt_[:], 0.0)
                nc.vector.tensor_copy(out=dt_[0:10, :], in_=o5[:, 0:512])
                nc.sync.dma_start(out=dbg[:], in_=dt_[:])
                continue
            nc.sync.dma_start(out=out[:, c * chunk : (c + 1) * chunk], in_=o5[:])

    nc.compile()
    return nc


def kernel(**inputs):
    inputs = {k: np.asarray(v) for k, v in inputs.items()}
    consts = build_consts(inputs)
    nc = build_nc(consts)
    xs = prep_x(inputs["x"].astype(np.float32))
    in_maps = [{"xt": xs[i]} for i in range(N_CORES)]
    res = run_bass_kernel_spmd(nc, in_maps, core_ids=list(range(N_CORES)))
    out = np.concatenate([np.asarray(r["out"]).astype(np.float32).T for r in res.results], axis=0)
    return out.astype(np.float32)
